# revision 8
# baseline (speedup 1.0000x reference)
"""Bidirectional attention kernel for Trainium2 (8 NeuronCores, SPMD).

Sharding: core = (batch b, feat-token quarter iq). Each core processes
NI = 8192 feat tokens for one batch across all 4 heads:
  - qv projection (PE), per-head scores in both orientations (PE),
  - exp via ScalarE (fused with PSUM evac),
  - feat-direction: Pfeat = MV2ext^T @ E^T  (rows 0-63: W_feat_out-projected
    head contribution, row 64: softmax row-sum),
  - map-direction: Uext = [v;1]^T @ E  (rows 0-63: U, row 64: denominator).
Host does the tiny map-side prep (map qv projection, MV2 precompute) and the
final normalization / head-sum / map output projection.
"""
import numpy as np

import concourse.bass as bass
import concourse.bacc as bacc
import concourse.mybir as mybir
import concourse.tile as tile
from concourse.bass import ts
from concourse.bass_utils import run_bass_kernel_spmd

F32 = mybir.dt.float32
EXP = mybir.ActivationFunctionType.Exp

B, C, H, D, M = 2, 64, 4, 64, 512
N = 32 * 32 * 32
SCALE = float(D) ** -0.5


def build_nc(NI=N // 4, stages="all"):
    IB = NI // 128            # 128-token i-blocks
    NG2 = NI // 512           # 512-token i-groups
    nc = bacc.Bacc("TRN2")

    f_d = nc.dram_tensor("f", [C, NI], F32, kind="ExternalInput")
    w_d = nc.dram_tensor("wqvT", [C, 512], F32, kind="ExternalInput")
    mq_d = nc.dram_tensor("mapq", [2, 128, M], F32, kind="ExternalInput")
    mv2_d = nc.dram_tensor("mv2e", [16, 128, 65], F32, kind="ExternalInput")
    pf_d = nc.dram_tensor("pf", [H, 65, NI], F32, kind="ExternalOutput")
    ue_d = nc.dram_tensor("ue", [H, 65, M], F32, kind="ExternalOutput")

    with tile.TileContext(nc) as tc:
        with (
            tc.tile_pool(name="const", bufs=1) as const_pool,
            tc.tile_pool(name="qp", bufs=1) as q_pool,
            tc.tile_pool(name="vep", bufs=2) as ve_pool,
            tc.tile_pool(name="enp", bufs=3) as en_pool,
            tc.tile_pool(name="etp", bufs=3) as et_pool,
            tc.tile_pool(name="outp", bufs=3) as out_pool,
            tc.tile_pool(name="ps_sc", bufs=3, space="PSUM") as sc_psum,
            tc.tile_pool(name="ps_acc", bufs=1, space="PSUM") as acc_psum,
        ):
            f_sb = const_pool.tile([C, NI], F32)
            w_sb = const_pool.tile([C, 512], F32)
            mq01 = const_pool.tile([128, M], F32)
            mq23 = const_pool.tile([128, M], F32)
            mv2_sb = const_pool.tile([128, 16, 65], F32)
            q01 = q_pool.tile([128, NI], F32)
            q23 = q_pool.tile([128, NI], F32)

            nc.sync.dma_start(f_sb[:, :], f_d[:, :])
            nc.sync.dma_start(w_sb[:, :], w_d[:, :])
            nc.sync.dma_start(mq01[:, :], mq_d[0])
            nc.sync.dma_start(mq23[:, :], mq_d[1])
            for t in range(16):
                nc.sync.dma_start(mv2_sb[:, t, :], mv2_d[t])

            # qv projection: q01 = [q_h0; q_h1], q23 = [q_h2; q_h3]
            for blk in range(2):
                qdst = q01 if blk == 0 else q23
                for c in range(NI // 512):
                    qv = sc_psum.tile([128, 2, 512], F32, tag="sc")
                    nc.tensor.matmul(qv[:, 0, :], w_sb[:, ts(blk, 128)],
                                     f_sb[:, ts(c, 512)], start=True, stop=True)
                    nc.vector.tensor_copy(qdst[:, ts(c, 512)], qv[:, 0, :])

            for h in (range(H) if stages != "qv" else []):
                off = (h % 2) * 64
                qsb = q01 if h < 2 else q23
                mqsb = mq01 if h < 2 else mq23

                # ---- prestage v^T (with ones column) for this head ----
                ve = ve_pool.tile([128, IB, 65], F32, tag="ve")
                nc.vector.memset(ve[:, :, 64], 1.0)
                for ib in range(IB):
                    vt = sc_psum.tile([128, 2, 512], F32, tag="sc")
                    nc.tensor.matmul(vt[:, 0, 0:64], f_sb[:, ts(ib, 128)],
                                     w_sb[:, 256 + h * 64: 320 + h * 64],
                                     start=True, stop=True)
                    nc.vector.tensor_copy(ve[:, ib, 0:64], vt[:, 0, 0:64])

                if stages == "vt":
                    continue
                # ---- pass 1: natural scores -> exp -> U accumulation ----
                u = acc_psum.tile([65, M], F32, tag="u")
                for g in range(IB // 2):
                    s1 = sc_psum.tile([128, 2, 512], F32, tag="sc")
                    for k in range(2):
                        ib = g * 2 + k
                        nc.tensor.matmul(s1[:, k, :],
                                         qsb[off:off + 64, ts(ib, 128)],
                                         mqsb[off:off + 64, :],
                                         start=True, stop=True)
                    en = en_pool.tile([128, 2, 512], F32, tag="en")
                    nc.scalar.activation(en[:, :, :], s1[:, :, :], EXP, scale=SCALE)
                    for k in range(2):
                        ib = g * 2 + k
                        nc.tensor.matmul(u[:, :], ve[:, ib, :], en[:, k, :],
                                         start=(ib == 0), stop=(ib == IB - 1))
                u_sb = out_pool.tile([65, M], F32, tag="u_sb")
                nc.vector.tensor_copy(u_sb[:, :], u[:, :])
                nc.sync.dma_start(ue_d[h], u_sb[:, :])

                if stages == "pass1":
                    continue
                # ---- pass 2: transposed scores -> exp -> Pfeat ----
                for g in range(NG2):
                    pf = acc_psum.tile([65, 512], F32, tag="pf")
                    for half in range(2):
                        st = sc_psum.tile([128, 2, 512], F32, tag="sc")
                        for q_ in range(2):
                            jb = half * 2 + q_
                            nc.tensor.matmul(st[:, q_, :],
                                             mqsb[off:off + 64, ts(jb, 128)],
                                             qsb[off:off + 64, ts(g, 512)],
                                             start=True, stop=True)
                        et = et_pool.tile([128, 2, 512], F32, tag="et")
                        nc.scalar.activation(et[:, :, :], st[:, :, :], EXP,
                                             scale=SCALE)
                        for q_ in range(2):
                            jb = half * 2 + q_
                            nc.tensor.matmul(pf[:, :], mv2_sb[:, h * 4 + jb, :],
                                             et[:, q_, :],
                                             start=(jb == 0), stop=(jb == 3))
                    pf_sb = out_pool.tile([65, 512], F32, tag="pf_sb")
                    nc.vector.tensor_copy(pf_sb[:, :], pf[:, :])
                    nc.sync.dma_start(pf_d[h, :, ts(g, 512)], pf_sb[:, :])

    nc.compile()
    return nc


_PERM = np.array([d * 4 + h for h in range(H) for d in range(D)])


def host_prep(feat, semantic_map, w_feat_qv, w_map_qv, w_feat_out, NI):
    f = feat.reshape(B, C, N)
    m = semantic_map.reshape(B, 128, M)
    wq = w_feat_qv[:256][_PERM]
    wv = w_feat_qv[256:][_PERM]
    wqvT = np.ascontiguousarray(np.concatenate([wq, wv], 0).T)

    mqv = np.einsum('oc,bcm->bom', w_map_qv, m)
    map_q = mqv[:, :256][:, _PERM].reshape(B, H, D, M)
    map_v = mqv[:, 256:][:, _PERM].reshape(B, H, D, M)
    wfo_h = w_feat_out[:, _PERM].reshape(64, H, D).transpose(1, 0, 2)  # (H,64,D)

    mapq_dev = np.ascontiguousarray(map_q.reshape(B, 2, 2, D, M)
                                    .reshape(B, 2, 128, M))
    mv2e = np.empty((B, H, M, 65), np.float32)
    for b in range(B):
        for h in range(H):
            mv2e[b, h, :, :64] = (wfo_h[h] @ map_v[b, h]).T
            mv2e[b, h, :, 64] = 1.0
    mv2_dev = np.ascontiguousarray(mv2e.reshape(B, H, 4, 128, 65)
                                   .reshape(B, 16, 128, 65))

    in_maps = []
    for core in range(8):
        b, iq = core // 4, core % 4
        in_maps.append({
            "f": np.ascontiguousarray(f[b, :, iq * NI:(iq + 1) * NI]),
            "wqvT": wqvT,
            "mapq": mapq_dev[b],
            "mv2e": mv2_dev[b],
        })
    return in_maps


def host_post(results, w_map_out, NI):
    wmo_h = w_map_out[:, _PERM].reshape(128, H, D).transpose(1, 0, 2)  # (H,128,D)
    feat_out = np.empty((B, 64, N), np.float32)
    U = np.zeros((B, H, D, M), np.float32)
    den = np.zeros((B, H, M), np.float32)
    for core in range(8):
        b, iq = core // 4, core % 4
        pf = results[core]["pf"]
        feat_out[b, :, iq * NI:(iq + 1) * NI] = \
            (pf[:, :64, :] / pf[:, 64:65, :]).sum(0)
        ue = results[core]["ue"]
        U[b] += ue[:, :64, :]
        den[b] += ue[:, 64, :]
    map_out = np.zeros((B, 128, M), np.float32)
    for b in range(B):
        for h in range(H):
            map_out[b] += wmo_h[h] @ (U[b, h] / den[b, h][None, :])
    return (feat_out.reshape(B, 64, 32, 32, 32),
            map_out.reshape(B, 128, 8, 8, 8))


_NC_CACHE = {}


def _get_nc(NI):
    if NI not in _NC_CACHE:
        _NC_CACHE[NI] = build_nc(NI)
    return _NC_CACHE[NI]


def kernel(feat, semantic_map, w_feat_qv, w_map_qv, w_feat_out, w_map_out,
           _trace=False):
    NI = N // 4
    feat = np.asarray(feat, np.float32)
    semantic_map = np.asarray(semantic_map, np.float32)
    w_feat_qv = np.asarray(w_feat_qv, np.float32)
    w_map_qv = np.asarray(w_map_qv, np.float32)
    w_feat_out = np.asarray(w_feat_out, np.float32)
    w_map_out = np.asarray(w_map_out, np.float32)

    nc = _get_nc(NI)
    in_maps = host_prep(feat, semantic_map, w_feat_qv, w_map_qv, w_feat_out, NI)
    res = run_bass_kernel_spmd(nc, in_maps, list(range(8)), trace=_trace)
    out = host_post(res.results, w_map_out, NI)
    if _trace:
        return out, res
    return out


# revision 9
# speedup vs baseline: 1.9646x; 1.9646x over previous
"""Bidirectional attention kernel for Trainium2 (8 NeuronCores, SPMD).

Sharding: core = (batch b, feat-token quarter iq). Each core processes
NI = 8192 feat tokens for one batch across all 4 heads:
  - qv projection (PE), per-head scores in both orientations (PE),
  - exp via ScalarE (fused with PSUM evac),
  - feat-direction: Pfeat = MV2ext^T @ E^T  (rows 0-63: W_feat_out-projected
    head contribution, row 64: softmax row-sum),
  - map-direction: Uext = [v;1]^T @ E  (rows 0-63: U, row 64: denominator).
Host does the tiny map-side prep (map qv projection, MV2 precompute) and the
final normalization / head-sum / map output projection.
"""
import numpy as np

import concourse.bass as bass
import concourse.bacc as bacc
import concourse.mybir as mybir
import concourse.tile as tile
from concourse.bass import ts
from concourse.bass_utils import run_bass_kernel_spmd

F32 = mybir.dt.float32
BF16 = mybir.dt.bfloat16
EXP = mybir.ActivationFunctionType.Exp

B, C, H, D, M = 2, 64, 4, 64, 512
N = 32 * 32 * 32
SCALE = float(D) ** -0.5


def build_nc(NI=N // 4, stages="all"):
    IB = NI // 128            # 128-token i-blocks
    NG2 = NI // 512           # 512-token i-groups
    nc = bacc.Bacc("TRN2")

    f_d = nc.dram_tensor("f", [C, NI], BF16, kind="ExternalInput")
    w_d = nc.dram_tensor("wqvT", [C, 512], BF16, kind="ExternalInput")
    mq_d = nc.dram_tensor("mapq", [2, 128, M], BF16, kind="ExternalInput")
    mv2_d = nc.dram_tensor("mv2e", [16, 128, 65], BF16, kind="ExternalInput")
    pf_d = nc.dram_tensor("pf", [H, 65, NI], F32, kind="ExternalOutput")
    ue_d = nc.dram_tensor("ue", [H, 65, M], F32, kind="ExternalOutput")

    with tile.TileContext(nc) as tc:
        with (
            tc.tile_pool(name="const", bufs=1) as const_pool,
            tc.tile_pool(name="qp", bufs=1) as q_pool,
            tc.tile_pool(name="vep", bufs=2) as ve_pool,
            tc.tile_pool(name="enp", bufs=3) as en_pool,
            tc.tile_pool(name="etp", bufs=3) as et_pool,
            tc.tile_pool(name="outp", bufs=3) as out_pool,
            tc.tile_pool(name="ps_sc", bufs=3, space="PSUM") as sc_psum,
            tc.tile_pool(name="ps_acc", bufs=1, space="PSUM") as acc_psum,
        ):
            f_sb = const_pool.tile([C, NI], BF16)
            w_sb = const_pool.tile([C, 512], BF16)
            mq01 = const_pool.tile([128, M], BF16)
            mq23 = const_pool.tile([128, M], BF16)
            mv2_sb = const_pool.tile([128, 16, 65], BF16)
            q01 = q_pool.tile([128, NI], BF16)
            q23 = q_pool.tile([128, NI], BF16)

            nc.sync.dma_start(f_sb[:, :], f_d[:, :])
            nc.sync.dma_start(w_sb[:, :], w_d[:, :])
            nc.sync.dma_start(mq01[:, :], mq_d[0])
            nc.sync.dma_start(mq23[:, :], mq_d[1])
            for t in range(16):
                nc.sync.dma_start(mv2_sb[:, t, :], mv2_d[t])

            # qv projection: q01 = [q_h0; q_h1], q23 = [q_h2; q_h3]
            for blk in range(2):
                qdst = q01 if blk == 0 else q23
                for c in range(NI // 512):
                    qv = sc_psum.tile([128, 2, 512], F32, tag="sc")
                    nc.tensor.matmul(qv[:, 0, :], w_sb[:, ts(blk, 128)],
                                     f_sb[:, ts(c, 512)], start=True, stop=True)
                    nc.vector.tensor_copy(qdst[:, ts(c, 512)], qv[:, 0, :])

            for h in (range(H) if stages != "qv" else []):
                off = (h % 2) * 64
                qsb = q01 if h < 2 else q23
                mqsb = mq01 if h < 2 else mq23

                # ---- prestage v^T (with ones column) for this head ----
                ve = ve_pool.tile([128, IB, 65], BF16, tag="ve")
                nc.vector.memset(ve[:, :, 64], 1.0)
                for ib in range(IB):
                    vt = sc_psum.tile([128, 2, 512], F32, tag="sc")
                    nc.tensor.matmul(vt[:, 0, 0:64], f_sb[:, ts(ib, 128)],
                                     w_sb[:, 256 + h * 64: 320 + h * 64],
                                     start=True, stop=True)
                    nc.vector.tensor_copy(ve[:, ib, 0:64], vt[:, 0, 0:64])

                if stages == "vt":
                    continue
                # ---- pass 1: natural scores -> exp -> U accumulation ----
                u = acc_psum.tile([65, M], F32, tag="u")
                for g in range(IB // 2):
                    s1 = sc_psum.tile([128, 2, 512], F32, tag="sc")
                    for k in range(2):
                        ib = g * 2 + k
                        nc.tensor.matmul(s1[:, k, :],
                                         qsb[off:off + 64, ts(ib, 128)],
                                         mqsb[off:off + 64, :],
                                         start=True, stop=True)
                    en = en_pool.tile([128, 2, 512], BF16, tag="en")
                    nc.scalar.activation(en[:, :, :], s1[:, :, :], EXP, scale=SCALE)
                    for k in range(2):
                        ib = g * 2 + k
                        nc.tensor.matmul(u[:, :], ve[:, ib, :], en[:, k, :],
                                         start=(ib == 0), stop=(ib == IB - 1))
                u_sb = out_pool.tile([65, M], F32, tag="u_sb")
                nc.vector.tensor_copy(u_sb[:, :], u[:, :])
                nc.sync.dma_start(ue_d[h], u_sb[:, :])

                if stages == "pass1":
                    continue
                # ---- pass 2: transposed scores -> exp -> Pfeat ----
                for g in range(NG2):
                    pf = acc_psum.tile([65, 512], F32, tag="pf")
                    for half in range(2):
                        st = sc_psum.tile([128, 2, 512], F32, tag="sc")
                        for q_ in range(2):
                            jb = half * 2 + q_
                            nc.tensor.matmul(st[:, q_, :],
                                             mqsb[off:off + 64, ts(jb, 128)],
                                             qsb[off:off + 64, ts(g, 512)],
                                             start=True, stop=True)
                        et = et_pool.tile([128, 2, 512], BF16, tag="et")
                        nc.scalar.activation(et[:, :, :], st[:, :, :], EXP,
                                             scale=SCALE)
                        for q_ in range(2):
                            jb = half * 2 + q_
                            nc.tensor.matmul(pf[:, :], mv2_sb[:, h * 4 + jb, :],
                                             et[:, q_, :],
                                             start=(jb == 0), stop=(jb == 3))
                    pf_sb = out_pool.tile([65, 512], F32, tag="pf_sb")
                    nc.vector.tensor_copy(pf_sb[:, :], pf[:, :])
                    nc.sync.dma_start(pf_d[h, :, ts(g, 512)], pf_sb[:, :])

    nc.compile()
    return nc


_PERM = np.array([d * 4 + h for h in range(H) for d in range(D)])


def host_prep(feat, semantic_map, w_feat_qv, w_map_qv, w_feat_out, NI):
    f = feat.reshape(B, C, N)
    m = semantic_map.reshape(B, 128, M)
    wq = w_feat_qv[:256][_PERM]
    wv = w_feat_qv[256:][_PERM]
    wqvT = np.ascontiguousarray(np.concatenate([wq, wv], 0).T)

    mqv = np.einsum('oc,bcm->bom', w_map_qv, m)
    map_q = mqv[:, :256][:, _PERM].reshape(B, H, D, M)
    map_v = mqv[:, 256:][:, _PERM].reshape(B, H, D, M)
    wfo_h = w_feat_out[:, _PERM].reshape(64, H, D).transpose(1, 0, 2)  # (H,64,D)

    mapq_dev = np.ascontiguousarray(map_q.reshape(B, 2, 2, D, M)
                                    .reshape(B, 2, 128, M))
    mv2e = np.empty((B, H, M, 65), np.float32)
    for b in range(B):
        for h in range(H):
            mv2e[b, h, :, :64] = (wfo_h[h] @ map_v[b, h]).T
            mv2e[b, h, :, 64] = 1.0
    mv2_dev = np.ascontiguousarray(mv2e.reshape(B, H, 4, 128, 65)
                                   .reshape(B, 16, 128, 65))

    import ml_dtypes
    bf16 = ml_dtypes.bfloat16
    in_maps = []
    for core in range(8):
        b, iq = core // 4, core % 4
        in_maps.append({
            "f": np.ascontiguousarray(f[b, :, iq * NI:(iq + 1) * NI]).astype(bf16),
            "wqvT": wqvT.astype(bf16),
            "mapq": mapq_dev[b].astype(bf16),
            "mv2e": mv2_dev[b].astype(bf16),
        })
    return in_maps


def host_post(results, w_map_out, NI):
    wmo_h = w_map_out[:, _PERM].reshape(128, H, D).transpose(1, 0, 2)  # (H,128,D)
    feat_out = np.empty((B, 64, N), np.float32)
    U = np.zeros((B, H, D, M), np.float32)
    den = np.zeros((B, H, M), np.float32)
    for core in range(8):
        b, iq = core // 4, core % 4
        pf = results[core]["pf"]
        feat_out[b, :, iq * NI:(iq + 1) * NI] = \
            (pf[:, :64, :] / pf[:, 64:65, :]).sum(0)
        ue = results[core]["ue"]
        U[b] += ue[:, :64, :]
        den[b] += ue[:, 64, :]
    map_out = np.zeros((B, 128, M), np.float32)
    for b in range(B):
        for h in range(H):
            map_out[b] += wmo_h[h] @ (U[b, h] / den[b, h][None, :])
    return (feat_out.reshape(B, 64, 32, 32, 32),
            map_out.reshape(B, 128, 8, 8, 8))


_NC_CACHE = {}


def _get_nc(NI):
    if NI not in _NC_CACHE:
        _NC_CACHE[NI] = build_nc(NI)
    return _NC_CACHE[NI]


def kernel(feat, semantic_map, w_feat_qv, w_map_qv, w_feat_out, w_map_out,
           _trace=False):
    NI = N // 4
    feat = np.asarray(feat, np.float32)
    semantic_map = np.asarray(semantic_map, np.float32)
    w_feat_qv = np.asarray(w_feat_qv, np.float32)
    w_map_qv = np.asarray(w_map_qv, np.float32)
    w_feat_out = np.asarray(w_feat_out, np.float32)
    w_map_out = np.asarray(w_map_out, np.float32)

    nc = _get_nc(NI)
    in_maps = host_prep(feat, semantic_map, w_feat_qv, w_map_qv, w_feat_out, NI)
    res = run_bass_kernel_spmd(nc, in_maps, list(range(8)), trace=_trace)
    out = host_post(res.results, w_map_out, NI)
    if _trace:
        return out, res
    return out
